# revision 1
# baseline (speedup 1.0000x reference)
"""Channel-transformer (CTR) attention kernel for Trainium2, 8 NeuronCores.

Problem: x (16, 256, 64, 64) f32, gamma scalar.
  xr = x.reshape(B, C, NH, DIM)                       # NH=8, DIM=512
  energy[b,h,c,k] = sum_d xr[b,c,h,d] * xr[b,k,h,d]   # symmetric (C x C)
  attn = softmax(rowmax(energy) - energy, axis=-1)    # == softmax(-energy)
  out[b,c,h,d] = sum_k attn[b,h,c,k] * xr[b,k,h,d]
  result = gamma * out + x

Sharding: data-parallel over batch, 2 samples per core; gamma replicated.

Per-core kernel (per batch b, head h):
  - keep x[b] resident in SBUF as two natural tiles X[m] = [128 ch, 4096]
  - XT (d-major) via 8 PE transposes of 128x128 blocks (f32)
  - E[m] = XT[:,m-half].T @ XT  (f32r matmuls, N=256, fp32 PSUM accumulate)
  - attnT[kc] = exp(-E[kc] - 64): the softmax max-shift cancels row-wise, so a
    constant bias suffices for range safety; E symmetric => E tiles are already
    the transposed-attention (k-major) layout the second matmul needs.
  - V[m] = sum_kc attnT[kc][:, m-half].T @ X[kc][:, head] (f32r, N=512)
    Z[m] = same weights against a ones column (row sums of unnormalized attn)
  - Y[m][:, head] = V[m] * (gamma / Z[m]) + X[m][:, head]  (one fused DVE op)
"""

import numpy as np

B, C, HW = 16, 256, 4096
NH, DIM = 8, 512
N_CORES = 8
BPC = B // N_CORES  # batches per core
EXP_BIAS = -64.0  # exp(-E + EXP_BIAS): keeps exponents < ~40 for N(0,1) inputs

_CACHE = {}


def _build_module():
    import os
    import concourse.bacc as bacc
    import concourse.tile as tile
    import concourse.mybir as mybir

    f32 = mybir.dt.float32
    bf16 = mybir.dt.bfloat16
    AF = mybir.ActivationFunctionType
    OP = mybir.AluOpType

    nc = bacc.Bacc("TRN2", target_bir_lowering=False, debug=False, num_devices=N_CORES)
    x_d = nc.dram_tensor("x", [BPC, C, HW], f32, kind="ExternalInput").ap()
    g_d = nc.dram_tensor("g", [1, 1], f32, kind="ExternalInput").ap()
    id_d = nc.dram_tensor("ident", [128, 128], bf16, kind="ExternalInput").ap()
    y_d = nc.dram_tensor("y", [BPC, C, HW], f32, kind="ExternalOutput").ap()

    with tile.TileContext(nc) as tc:
        from contextlib import ExitStack

        with ExitStack() as ctx:
            x_pool = ctx.enter_context(tc.tile_pool(name="xs", bufs=2 * BPC))
            xb_pool = ctx.enter_context(tc.tile_pool(name="xb", bufs=2 * BPC))
            st_pool = ctx.enter_context(tc.tile_pool(name="st", bufs=8))
            xt_pool = ctx.enter_context(tc.tile_pool(name="xt", bufs=6))
            at_pool = ctx.enter_context(tc.tile_pool(name="at", bufs=8))
            r_pool = ctx.enter_context(tc.tile_pool(name="rp", bufs=24))
            _tp = int(os.environ.get("K_TP", "2"))
            _pe = int(os.environ.get("K_PE", "2"))
            _pv = int(os.environ.get("K_PV", "4"))
            tp_pool = ctx.enter_context(tc.tile_pool(name="tp", bufs=_tp, space="PSUM"))
            e_pool = ctx.enter_context(tc.tile_pool(name="pe", bufs=_pe, space="PSUM"))
            ev_pool = ctx.enter_context(tc.tile_pool(name="ev", bufs=_pv, space="PSUM"))

            cpool = ctx.enter_context(tc.tile_pool(name="const", bufs=1))
            ident = cpool.tile([128, 128], bf16)
            nc.sync.dma_start(ident[:], id_d[:])
            ebias = cpool.tile([128, 1], f32)
            nc.gpsimd.memset(ebias[:], EXP_BIAS)
            onesr = cpool.tile([1, 128], f32)
            nc.gpsimd.memset(onesr[:], 1.0)
            gsb = cpool.tile([1, 1], f32)
            nc.sync.dma_start(gsb[:], g_d[:])
            gamma128 = cpool.tile([128, 1], f32)
            # broadcast gamma to all partitions: [128,1] = ones[1,128].T @ g[1,1]
            gps = ev_pool.tile([128, 1], f32, tag="ev", name="gps")
            nc.tensor.matmul(gps[:], onesr[:], gsb[:], start=True, stop=True)
            nc.scalar.copy(gamma128[:], gps[:])

            warm = e_pool.tile([128, 512], f32, tag="pe", name="warm")
            for _w in range(56):
                nc.tensor.matmul(warm[0:64, 0:64], ident[0:64, 0:64], ident[0:64, 0:64], start=True, stop=True)

            CH = 1024  # load/cast chunk (columns)
            Xall, XBall = [], []
            for b in range(BPC):
                X = [x_pool.tile([128, HW], f32, tag="xs", name=f"X{b}_{m}") for m in range(2)]
                XB = [xb_pool.tile([128, HW], bf16, tag="xb", name=f"XB{b}_{m}") for m in range(2)]
                for c0 in range(0, HW, CH):
                    for m in range(2):
                        nc.sync.dma_start(
                            X[m][:, c0 : c0 + CH],
                            x_d[b, 128 * m : 128 * (m + 1), c0 : c0 + CH],
                        )
                        nc.vector.tensor_copy(XB[m][:, c0 : c0 + CH], X[m][:, c0 : c0 + CH])
                Xall.append(X)
                XBall.append(XB)



            for b in range(BPC):
                X, XB = Xall[b], XBall[b]

                for h in range(NH):
                    col = DIM * h
                    # ---- XT = [128 d, 4*256 ch] bf16: 8 PE transposes packed
                    # into one PSUM bank, then one wide copy to SBUF ----
                    tp = tp_pool.tile([128, 1024], bf16, tag="tp", name=f"TP{b}_{h}")
                    for kd in range(4):
                        for m in range(2):
                            nc.tensor.transpose(
                                tp[:, 256 * kd + 128 * m : 256 * kd + 128 * (m + 1)],
                                XB[m][:, col + 128 * kd : col + 128 * (kd + 1)],
                                ident[:],
                            )
                    XT = xt_pool.tile([128, 1024], bf16, tag="xt", name=f"XT{b}_{h}")
                    nc.scalar.copy(XT[:], tp[:])

                    # ---- E[:, m-half] = XT[:, m-half].T @ XT (accumulate over kd) ----
                    E = e_pool.tile([128, 512], f32, tag="pe", name=f"E{b}_{h}")
                    for m in range(2):
                        for kd in range(4):
                            nc.tensor.matmul(
                                E[:, 256 * m : 256 * (m + 1)],
                                XT[:, 256 * kd + 128 * m : 256 * kd + 128 * (m + 1)],
                                XT[:, 256 * kd : 256 * (kd + 1)],
                                start=(kd == 0),
                                stop=(kd == 3),
                            )

                    # ---- attnT[kc] = exp(-E - 64)  (bf16); E symmetric, so this
                    # tile is unnormalized-attn^T with k on partitions, and its
                    # per-partition row sum (accum_out) equals the softmax
                    # denominator Z for channel block kc ----
                    AT = []
                    Zp = r_pool.tile([128, 2], f32, tag="rp", name=f"Zp{b}_{h}")
                    for kc in range(2):
                        a = at_pool.tile([128, 256], bf16, tag="at", name=f"AT{b}_{h}_{kc}")
                        nc.scalar.activation(
                            a[:], E[:, 256 * kc : 256 * (kc + 1)], AF.Exp, scale=-1.0, bias=ebias[:], accum_out=Zp[:, kc : kc + 1]
                        )
                        AT.append(a)

                    # ---- V[m] += attnT[kc][:, m-half].T @ XB[kc][:, head] ----
                    V = [ev_pool.tile([128, DIM], f32, tag="ev", name=f"V{b}_{h}_{m}") for m in range(2)]
                    for m in range(2):
                        for kc in range(2):
                            nc.tensor.matmul(
                                V[m][:],
                                AT[kc][:, 128 * m : 128 * (m + 1)],
                                XB[kc][:, col : col + DIM],
                                start=(kc == 0),
                                stop=(kc == 1),
                            )

                    # ---- out = V * (gamma / Z) + X[m][:, head]; store per head ----
                    Rp = r_pool.tile([128, 2], f32, tag="rp", name=f"Rp{b}_{h}")
                    nc.vector.reciprocal(Rp[:], Zp[:])
                    gRp = r_pool.tile([128, 2], f32, tag="rp", name=f"gRp{b}_{h}")
                    nc.gpsimd.tensor_scalar(gRp[:], Rp[:], gamma128[:], None, op0=OP.mult)
                    for m in range(2):
                        st = st_pool.tile([128, DIM], f32, tag="st", name=f"ST{b}_{h}_{m}")
                        nc.vector.scalar_tensor_tensor(
                            st[:],
                            V[m][:],
                            gRp[:, m : m + 1],
                            X[m][:, col : col + DIM],
                            op0=OP.mult,
                            op1=OP.add,
                        )
                        nc.sync.dma_start(
                            y_d[b, 128 * m : 128 * (m + 1), col : col + DIM], st[:]
                        )

    nc.compile()
    return nc


def _get_module():
    if "nc" not in _CACHE:
        _CACHE["nc"] = _build_module()
    return _CACHE["nc"]


def kernel(x_input, gamma):
    from concourse.bass_utils import run_bass_kernel_spmd

    nc = _get_module()

    x = np.ascontiguousarray(np.asarray(x_input, dtype=np.float32)).reshape(B, C, HW)
    g = np.asarray(gamma, dtype=np.float32).reshape(1, 1)
    import ml_dtypes
    ident = np.eye(128, dtype=ml_dtypes.bfloat16)

    in_maps = [
        {
            "x": np.ascontiguousarray(x[i * BPC : (i + 1) * BPC]),
            "g": g,
            "ident": ident,
        }
        for i in range(N_CORES)
    ]
    res = run_bass_kernel_spmd(nc, in_maps, list(range(N_CORES)))
    y = np.concatenate([res.results[i]["y"] for i in range(N_CORES)], axis=0)
    return y.reshape(B, C, 64, 64).astype(np.float32)



# revision 2
# speedup vs baseline: 1.1672x; 1.1672x over previous
"""Channel-transformer (CTR) attention kernel for Trainium2, 8 NeuronCores.

Problem: x (16, 256, 64, 64) f32, gamma scalar.
  xr = x.reshape(B, C, NH, DIM)                       # NH=8, DIM=512
  energy[b,h,c,k] = sum_d xr[b,c,h,d] * xr[b,k,h,d]   # symmetric (C x C)
  attn = softmax(rowmax(energy) - energy, axis=-1)    # == softmax(-energy)
  out[b,c,h,d] = sum_k attn[b,h,c,k] * xr[b,k,h,d]
  result = gamma * out + x
Sharding: data-parallel over batch, 2 samples per core; gamma replicated.

v2 design (per core, all I/O in bf16; host casts f32<->bf16):
  - host ships x twice: c-major XB tiles [128ch, 4096] and d-major XT tiles
    [128d, (h01,kd,c)] packed per head-pair so every DMA row is 4KB.
    (K_XBAR=1 instead builds XT on-device with DMA XBAR transposes.)
  - PE does only the attention math: per (b,h) 8 E matmuls (N=256, bf16)
    and 4 V matmuls (N=512), stream-time floor 27.3us/core.
  - softmax: attnT[kc] = exp(-E - 64) on Scalar with accum_out giving the
    row sums Z (max-shift cancels row-wise; constant bias for range safety;
    E symmetric => exp tile is already k-major for the V matmul).
  - V accumulation order (m0kc0, m1kc0, m0kc1, m1kc1) hides the exp(kc=1)
    latency behind the first two V matmuls.
  - Y accumulated in SBUF bf16, stored per (b, head-pair, m): 2KB rows,
    tail is only the last pair's 512KB.
  - PE warmup matmuls run on a memset tile: no DMA dependency, so the
    clock ramp overlaps the first loads.
"""

import os
import numpy as np

B, C, HW = 16, 256, 4096
NH, DIM = 8, 512
N_CORES = 8
BPC = B // N_CORES  # batches per core
EXP_BIAS = -64.0  # exp(-E + EXP_BIAS): keeps exponents < ~85 for N(0,1) inputs

_CACHE = {}


def _build_module():
    import concourse.bacc as bacc
    import concourse.tile as tile
    import concourse.mybir as mybir

    f32 = mybir.dt.float32
    bf16 = mybir.dt.bfloat16
    AF = mybir.ActivationFunctionType
    OP = mybir.AluOpType

    use_xbar = os.environ.get("K_XBAR", "0") == "1"
    n_warm = int(os.environ.get("K_WARM", "24"))
    _pe = int(os.environ.get("K_PE", "3"))
    _pv = int(os.environ.get("K_PV", "4"))
    _xtb = int(os.environ.get("K_XTB", "3"))

    nc = bacc.Bacc("TRN2", target_bir_lowering=False, debug=False, num_devices=N_CORES)
    xb_d = nc.dram_tensor("xb", [BPC, 2, 128, HW], bf16, kind="ExternalInput").ap()
    if not use_xbar:
        xt_d = nc.dram_tensor("xt", [BPC, 4, 128, 2048], bf16, kind="ExternalInput").ap()
    g_d = nc.dram_tensor("g", [1, 1], f32, kind="ExternalInput").ap()
    y_d = nc.dram_tensor("y", [BPC, 2, 128, HW], bf16, kind="ExternalOutput").ap()

    with tile.TileContext(nc) as tc:
        from contextlib import ExitStack

        with ExitStack() as ctx:
            xb_pool = ctx.enter_context(tc.tile_pool(name="xb", bufs=2 * BPC))
            xt_pool = ctx.enter_context(tc.tile_pool(name="xt", bufs=_xtb))
            y_pool = ctx.enter_context(tc.tile_pool(name="ys", bufs=2 * BPC))
            at_pool = ctx.enter_context(tc.tile_pool(name="at", bufs=4))
            r_pool = ctx.enter_context(tc.tile_pool(name="rp", bufs=12))
            e_pool = ctx.enter_context(tc.tile_pool(name="pe", bufs=_pe, space="PSUM"))
            v_pool = ctx.enter_context(tc.tile_pool(name="pv", bufs=_pv, space="PSUM"))

            cpool = ctx.enter_context(tc.tile_pool(name="const", bufs=1))
            ebias = cpool.tile([128, 1], f32)
            nc.gpsimd.memset(ebias[:], EXP_BIAS)
            onesr = cpool.tile([1, 128], f32)
            nc.gpsimd.memset(onesr[:], 1.0)
            wz = cpool.tile([128, 128], bf16)
            nc.gpsimd.memset(wz[:], 0.0)
            gsb = cpool.tile([1, 1], f32)
            nc.sync.dma_start(gsb[:], g_d[:])
            gamma128 = cpool.tile([128, 1], f32)
            # broadcast gamma to all partitions: [128,1] = ones[1,128].T @ g[1,1]
            gps = v_pool.tile([128, 1], f32, tag="pv", name="gps")
            nc.tensor.matmul(gps[:], onesr[:], gsb[:], start=True, stop=True)
            nc.scalar.copy(gamma128[:], gps[:])

            # PE clock warmup on the zero tile (no DMA dependency)
            warm = e_pool.tile([128, 512], f32, tag="pe", name="warm")
            for _w in range(n_warm):
                nc.tensor.matmul(warm[0:64, 0:64], wz[0:64, 0:64], wz[0:64, 0:64], start=True, stop=True)

            # ---- loads: xt(b,hp) interleaved with xb halves so head 0's
            # operands land first and loads stay just ahead of the PE ----
            XB = [[xb_pool.tile([128, HW], bf16, tag="xb", name=f"XB{b}_{m}") for m in range(2)] for b in range(BPC)]
            XT2 = {}
            Y = [[y_pool.tile([128, HW], bf16, tag="ys", name=f"Y{b}_{m}") for m in range(2)] for b in range(BPC)]

            def load_xt(b, hp):
                t = xt_pool.tile([128, 2, 4, 256], bf16, tag="xt", name=f"XT{b}_{hp}")
                XT2[(b, hp)] = t
                if use_xbar:
                    for h01 in range(2):
                        col = DIM * (2 * hp + h01)
                        nc.sync.dma_start(
                            t[:, h01],
                            xb_d[b, :, :, col : col + DIM],
                            transpose=True,
                        )
                else:
                    nc.sync.dma_start(t[:], xt_d[b, hp])
                return t

            for b in range(BPC):
                load_xt(b, 0)
                for m in range(2):
                    nc.sync.dma_start(XB[b][m][:, 0:2048], xb_d[b, m, :, 0:2048])
                load_xt(b, 1)
                load_xt(b, 2)
                for m in range(2):
                    nc.sync.dma_start(XB[b][m][:, 2048:4096], xb_d[b, m, :, 2048:4096])
                load_xt(b, 3)

            for b in range(BPC):
                for h in range(NH):
                    hp, h01 = h // 2, h % 2
                    col = DIM * h
                    XT = XT2[(b, hp)]

                    # ---- E[:, 256m + k] = energy[c=128m+p, k], accumulated
                    # over the 4 d-chunks; lhsT/rhs both from the d-major tile ----
                    E = e_pool.tile([128, 512], f32, tag="pe", name=f"E{b}_{h}")
                    for m in range(2):
                        for kd in range(4):
                            nc.tensor.matmul(
                                E[:, 256 * m : 256 * (m + 1)],
                                XT[:, h01, kd, 128 * m : 128 * (m + 1)],
                                XT[:, h01, kd],
                                start=(kd == 0),
                                stop=(kd == 3),
                            )

                    # ---- attnT[kc] = exp(-E - 64) (bf16); accum_out = Z ----
                    AT = []
                    Zp = r_pool.tile([128, 2], f32, tag="rp", name=f"Zp{b}_{h}")
                    for kc in range(2):
                        a = at_pool.tile([128, 256], bf16, tag="at", name=f"AT{b}_{h}_{kc}")
                        nc.scalar.activation(
                            a[:], E[:, 256 * kc : 256 * (kc + 1)], AF.Exp,
                            scale=-1.0, bias=ebias[:], accum_out=Zp[:, kc : kc + 1],
                        )
                        AT.append(a)

                    # ---- V[m] += attnT[kc][:, m-half].T @ XB[kc][:, head];
                    # kc-major order so exp(kc=1) hides behind the first two ----
                    V = [v_pool.tile([128, DIM], f32, tag="pv", name=f"V{b}_{h}_{m}") for m in range(2)]
                    for kc in range(2):
                        for m in range(2):
                            nc.tensor.matmul(
                                V[m][:],
                                AT[kc][:, 128 * m : 128 * (m + 1)],
                                XB[b][kc][:, col : col + DIM],
                                start=(kc == 0),
                                stop=(kc == 1),
                            )

                    # ---- Y[m][:, head] = V * (gamma / Z) + XB[m][:, head] ----
                    Rp = r_pool.tile([128, 2], f32, tag="rp", name=f"Rp{b}_{h}")
                    nc.vector.reciprocal(Rp[:], Zp[:])
                    gRp = r_pool.tile([128, 2], f32, tag="rp", name=f"gRp{b}_{h}")
                    nc.gpsimd.tensor_scalar(gRp[:], Rp[:], gamma128[:], None, op0=OP.mult)
                    for m in range(2):
                        nc.vector.scalar_tensor_tensor(
                            Y[b][m][:, col : col + DIM],
                            V[m][:],
                            gRp[:, m : m + 1],
                            XB[b][m][:, col : col + DIM],
                            op0=OP.mult,
                            op1=OP.add,
                        )

                    # ---- store per head-pair: 2KB rows, small final tail ----
                    if h01 == 1:
                        for m in range(2):
                            nc.sync.dma_start(
                                y_d[b, m, :, 1024 * hp : 1024 * (hp + 1)],
                                Y[b][m][:, 1024 * hp : 1024 * (hp + 1)],
                            )

    nc.compile()
    return nc


def _get_module():
    if "nc" not in _CACHE:
        _CACHE["nc"] = _build_module()
    return _CACHE["nc"]


def _make_in_maps(x_np, g_np):
    """Shard + pack FULL inputs into the per-core DRAM tensors (bf16)."""
    import ml_dtypes

    bf16 = ml_dtypes.bfloat16
    x = np.ascontiguousarray(np.asarray(x_np, dtype=np.float32)).reshape(B, C, HW)
    xbf = x.astype(bf16)
    xb = np.ascontiguousarray(xbf.reshape(B, 2, 128, HW))
    g = np.asarray(g_np, dtype=np.float32).reshape(1, 1)

    use_xbar = os.environ.get("K_XBAR", "0") == "1"
    if use_xbar:
        xt = None
    else:
        # xt[b, hp, p, h01, kd, c] = x[b, c, 512*(2hp+h01) + 128kd + p]
        xtv = xbf.reshape(B, C, NH // 2, 2, 4, 128)
        xt = np.ascontiguousarray(xtv.transpose(0, 2, 5, 3, 4, 1)).reshape(B, NH // 2, 128, 2048)

    maps = []
    for i in range(N_CORES):
        m = {
            "xb": np.ascontiguousarray(xb[i * BPC : (i + 1) * BPC]),
            "g": g,
        }
        if xt is not None:
            m["xt"] = np.ascontiguousarray(xt[i * BPC : (i + 1) * BPC])
        maps.append(m)
    return maps


def kernel(x_input, gamma):
    from concourse.bass_utils import run_bass_kernel_spmd

    nc = _get_module()
    in_maps = _make_in_maps(x_input, gamma)
    res = run_bass_kernel_spmd(nc, in_maps, list(range(N_CORES)))
    y = np.concatenate([np.asarray(res.results[i]["y"]) for i in range(N_CORES)], axis=0)
    # y is [B, 2, 128, HW] bf16 with channels = 128*m + p
    return y.reshape(B, C, 64, 64).astype(np.float32)
